# revision 1
# baseline (speedup 1.0000x reference)
"""Trainium2 Bass kernel for masked multi-head attention block (qkv proj +
softmax(QK^T/sqrt(hd)) with boolean mask + AV + output proj).

Sharding (8 cores): core c -> batch b=c//2, q-row chunk r=c%2 (1024 q rows).
Each core computes k/v for the full 2048 rows of its batch (redundant x2),
attention + output projection for its 1024 q rows. No collectives; outputs
are disjoint. Host pre-transposes x and weights (layout prep only) and
permutes sequence columns so every core runs the identical program.

On-chip layout is feature-major: T(x)=[cin, seq], T(q/k)=[head dims, seq].
S^T tiles [k_seq, q_seq] come from lhsT=T(k) slices, rhs=T(q); softmax runs
without max subtraction (logits here are O(3); exp cannot overflow), mask is
applied multiplicatively after exp (host feeds keep-mask = ~mask as bf16).
AV uses lhsT=[V | ones] so PSUM row 64 accumulates softmax denominators.
All matmuls are bf16 (fp32r measured slower on HW; fp8 would blow the
accuracy budget), accumulation fp32 in PSUM.

Structure (the kernel is PE-stream-bound; every matmul costs out-cols
cycles and every Ldweights burns PE dispatch):
 - v proj entirely in the prologue, both 512-wide v chunks per xt Ldweights;
   pair-0 q/k proj is emitted first each rep so its tiny weight DMAs cover
   the wv DMA (wv and wp time-share one SBUF buffer via pool rotation).
 - attention is kt-outer: one K lds serves both 512-col q chunks of a head,
   one V lds serves both AV matmuls, and AV (lagged LAG kt behind S, with
   the softmax chain in between) interleaves S at head granularity.
 - softmax: exp on Act + mask-multiply split evenly between DVE (from PSUM)
   and Pool (bf16 in SBUF) so no single elementwise engine saturates.
 - per-pair normalization (reciprocal of the ones-row, PE broadcast matmul,
   multiply into ao) is deferred into the next pair's attention loop so its
   latency never stalls the in-order PE stream.
 - out proj keeps W_p stationary (ct-outer, lds shared across seq chunks),
   producing y^T in DRAM; the host transposes back.
 - PSUM: psS 3 (S + bcast + out-proj) + psAV 4 (AV accum) + psP 1 (proj).
"""

from contextlib import ExitStack

import numpy as np

import concourse.bass as bass
import concourse.tile as tile
from concourse import bacc, mybir

F32 = mybir.dt.float32
BF16 = mybir.dt.bfloat16
F32R = mybir.dt.float32r
Exp = mybir.ActivationFunctionType.Exp

P = 128


class Dims:
    def __init__(self, S, SQ, C, H, HD=64):
        self.S, self.SQ, self.C, self.H, self.HD = S, SQ, C, H, HD
        self.CT = C // P            # cin tiles
        self.NPAIR = H // 2         # head pairs
        self.KT = S // P            # k seq tiles
        self.QCW = min(512, SQ)     # q chunk width
        self.QC = SQ // self.QCW    # q chunks
        self.VH = min(8, H)         # heads per v chunk
        self.VCH = H // self.VH     # v chunks
        self.VW = self.VH * (HD + 1)  # v chunk cols incl ones col
        self.ST = SQ // P           # q seq tiles for proj
        assert H % 2 == 0 and C % P == 0 and S % P == 0
        assert self.NPAIR % (self.VH // 2) == 0


FULL = Dims(S=2048, SQ=1024, C=1024, H=16, HD=64)


def r(ap):
    return ap.bitcast(F32R)


def emit_body(ctx, tc, d, io, rep=1):
    nc = tc.nc
    HD = d.HD
    xT_h, wqkT_h, wvT_h, wpT_h, bqk_h, bv_h, bp_h, maskT_h, yT_h = io
    ctx.enter_context(nc.allow_low_precision(
        reason="bf16 matmul pipeline; accumulation stays fp32 in PSUM"))

    const = ctx.enter_context(tc.tile_pool(name="const", bufs=1))
    ones_f32 = const.tile([P, d.HD], F32)
    nc.vector.memset(ones_f32[:], 1.0)
    ones_row = const.tile([P, d.HD], F32R)
    nc.vector.tensor_copy(ones_row[:], ones_f32[:])
    ones_bf = const.tile([P, max(d.HD, d.KT)], BF16)
    nc.vector.memset(ones_bf[:], 1.0)
    # biases: bqk_sb[:, j] = bqk[j*128 : (j+1)*128]; bp_col likewise
    bqk_sb = const.tile([P, 2 * d.CT], F32)
    nc.sync.dma_start(out=bqk_sb[:], in_=bqk_h[:].rearrange("(j p) -> p j", p=P))
    bp_col = const.tile([P, d.CT], F32)
    nc.sync.dma_start(out=bp_col[:], in_=bp_h[:].rearrange("(j p) -> p j", p=P))
    bias_pool = ctx.enter_context(tc.tile_pool(name="bias_pool", bufs=1))
    bv_ap = bv_h[:]
    bv_bcast = bias_pool.tile([P, d.C], F32, tag="bias")
    nc.sync.dma_start(
        out=bv_bcast[:],
        in_=bass.AP(tensor=bv_ap.tensor, offset=bv_ap.offset, ap=[[0, P]] + list(bv_ap.ap)),
    )

    ao_pool = ctx.enter_context(tc.tile_pool(name="ao_pool", bufs=1))
    ao = ao_pool.tile([P, d.CT, d.SQ], BF16)
    # PSUM: 3 (S + bcast + out-proj) + 4 (AV accum) + 1 (proj) = 8 banks
    psS = ctx.enter_context(tc.tile_pool(name="psS", bufs=3, space="PSUM"))
    psAV = ctx.enter_context(tc.tile_pool(name="psAV", bufs=4, space="PSUM"))
    psP = ctx.enter_context(tc.tile_pool(name="psP", bufs=1, space="PSUM"))

    mask_pool = ctx.enter_context(tc.tile_pool(name="mask_pool", bufs=1))
    xt_pool = ctx.enter_context(tc.tile_pool(name="xt_pool", bufs=1))
    wqk_pool = ctx.enter_context(tc.tile_pool(name="wqk_pool", bufs=2))
    # wv (prologue) and wp (epilogue) time-share one [P, CT, C] buffer
    wbig_pool = ctx.enter_context(tc.tile_pool(name="wbig_pool", bufs=1))
    qk_pool = ctx.enter_context(tc.tile_pool(name="qk_pool", bufs=2))
    vaug_pool = ctx.enter_context(tc.tile_pool(name="vaug_pool", bufs=1))
    p_pool = ctx.enter_context(tc.tile_pool(name="p_pool", bufs=10))
    small_pool = ctx.enter_context(tc.tile_pool(name="small_pool", bufs=2))
    y_pool = ctx.enter_context(tc.tile_pool(name="y_pool", bufs=2))

    # inputs staged once, reused by every rep
    mT = mask_pool.tile([P, d.KT, d.SQ], BF16)
    maskT_r = maskT_h[:].rearrange("(t p) q -> p t q", p=P)
    for kt in range(d.KT):
        nc.sync.dma_start(out=mT[:, kt, :], in_=maskT_r[:, kt, :])
    xt = xt_pool.tile([P, d.CT, d.S], BF16)
    xT_r = xT_h[:].rearrange("(t p) s -> p t s", p=P)
    for ct in range(d.CT):
        nc.sync.dma_start(out=xt[:, ct, :], in_=xT_r[:, ct, :])

    wqkT_r = wqkT_h[:].rearrange("(t p) c -> p t c", p=P)
    wvT_r = wvT_h[:].rearrange("(t p) c -> p t c", p=P)
    wpT_r = wpT_h[:].rearrange("(t p) c -> p t c", p=P)

    LAG = 4  # kt iterations the AV matmuls trail the S matmuls by

    for rep_i in range(rep):
        # ---- prologue. q/k of pair 0 runs first: its weights are tiny DMAs,
        # so it covers the wv DMA latency (wv's buffer is freed by the
        # previous rep's out-proj only at the rep boundary).
        v_sb = vaug_pool.tile([P, d.KT, d.VCH * d.VW], BF16, tag="v_sb")
        wv_all = wbig_pool.tile([P, d.CT, d.C], BF16, tag="w", name="wv")
        for ct in range(d.CT):
            nc.sync.dma_start(out=wv_all[:, ct, :], in_=wvT_r[:, ct, :])
        qk_tiles = {}

        def gen_proj(pair):
            """Stepwise emitter for pair's q/k projections."""
            wq_p = wqk_pool.tile([P, d.CT, P], BF16, name="wq_p")
            nc.sync.dma_start(out=wq_p[:], in_=wqkT_r[:, :, pair * P:(pair + 1) * P])
            wk_p = wqk_pool.tile([P, d.CT, P], BF16, name="wk_p")
            nc.sync.dma_start(out=wk_p[:], in_=wqkT_r[:, :, d.C + pair * P:d.C + (pair + 1) * P])
            q_sb = qk_pool.tile([P, d.SQ], BF16, name="q_sb")
            k_sb = qk_pool.tile([P, d.S], BF16, name="k_sb")
            qk_tiles[pair] = (q_sb, k_sb)
            yield
            for qc in range(d.QC):
                psq = psP.tile([P, d.QCW], F32, tag="ps", name="psq")
                for ct in range(d.CT):
                    nc.tensor.matmul(
                        psq[:], wq_p[:, ct, :], xt[:, ct, qc * d.QCW:(qc + 1) * d.QCW],
                        start=(ct == 0), stop=(ct == d.CT - 1))
                nc.vector.tensor_scalar_add(q_sb[:, qc * d.QCW:(qc + 1) * d.QCW], psq[:],
                                             bqk_sb[:, pair:pair + 1])
                yield
            for kc in range(d.S // 512):
                psk = psP.tile([P, 512], F32, tag="ps", name="psk")
                for ct in range(d.CT):
                    nc.tensor.matmul(
                        psk[:], wk_p[:, ct, :], xt[:, ct, kc * 512:(kc + 1) * 512],
                        start=(ct == 0), stop=(ct == d.CT - 1))
                nc.vector.tensor_scalar_add(k_sb[:, kc * 512:(kc + 1) * 512], psk[:],
                                             bqk_sb[:, d.CT + pair:d.CT + pair + 1])
                yield

        # prologue: project pair 0 fully, then V for all heads (both 512-col
        # chunks share one xt lds per (st, ct)).
        for _ in gen_proj(0):
            pass
        for h in range(d.H):
            chunk, hh = divmod(h, d.VH)
            cc = chunk * d.VW + hh * (HD + 1) + HD
            nc.vector.tensor_copy(
                v_sb[:, :, cc:cc + 1].rearrange("p t x -> p (t x)"),
                ones_bf[:, 0:d.KT])
        for st in range(d.KT):
            psv = [psS.tile([P, d.VH * HD], F32, tag="ps", name=f"psv{c}")
                   for c in range(d.VCH)]
            for ct in range(d.CT):
                for chunk in range(d.VCH):
                    nc.tensor.matmul(
                        psv[chunk][:], xt[:, ct, st * P:(st + 1) * P],
                        wv_all[:, ct, chunk * d.VH * HD:(chunk + 1) * d.VH * HD],
                        start=(ct == 0), stop=(ct == d.CT - 1))
            for chunk in range(d.VCH):
                c0 = chunk * d.VH * HD
                dst = v_sb[:, st, chunk * d.VW:(chunk + 1) * d.VW].rearrange(
                    "p (h x) -> p h x", x=HD + 1)[:, :, 0:HD]
                nc.vector.tensor_tensor(
                    dst, psv[chunk][:].rearrange("p (h x) -> p h x", x=HD),
                    bv_bcast[:, c0:c0 + d.VH * HD].rearrange("p (h x) -> p h x", x=HD),
                    mybir.AluOpType.add)

        # wp reuses wv's buffer; the DMA waits until the last v-proj matmul
        # has read wv, and wp itself is only read by the epilogue.
        wp_sb = wbig_pool.tile([P, d.CT, d.C], BF16, tag="w", name="wp")
        for ct in range(d.CT):
            nc.sync.dma_start(out=wp_sb[:, ct, :], in_=wpT_r[:, ct, :])

        def norm_steps(pair, av):
            """Deferred normalization of pair's AV accumulators into ao.
            Three yield-separated stages so the PE broadcast matmul is
            emitted well after its reciprocal, never stalling the stream."""
            recips = []
            for h01 in range(2):
                recip = small_pool.tile([1, d.SQ], F32R, tag="recip", name="recip")
                for qc in range(d.QC):
                    nc.vector.reciprocal(recip[:, qc * d.QCW:(qc + 1) * d.QCW],
                                         av[h01][qc][HD:HD + 1, :])
                recips.append(recip)
            yield
            bcs = []
            for h01 in range(2):
                bc_sb = small_pool.tile([HD, d.SQ], BF16, tag="bc", name="bc_sb")
                for qc in range(d.QC):
                    bc_ps = psS.tile([HD, d.QCW], F32, tag="ps", name="bc_ps")
                    nc.tensor.matmul(bc_ps[:], r(ones_row[0:1, 0:HD]),
                                     r(recips[h01][:, qc * d.QCW:(qc + 1) * d.QCW]),
                                     start=True, stop=True)
                    nc.vector.tensor_copy(bc_sb[:, qc * d.QCW:(qc + 1) * d.QCW],
                                          bc_ps[:])
                bcs.append(bc_sb)
            yield
            for h01 in range(2):
                for qc in range(d.QC):
                    nc.vector.tensor_tensor(
                        ao[h01 * HD:(h01 + 1) * HD, pair, qc * d.QCW:(qc + 1) * d.QCW],
                        av[h01][qc][0:HD, :], bcs[h01][:, qc * d.QCW:(qc + 1) * d.QCW],
                        mybir.AluOpType.mult)
            yield

        pend_norm = None
        for pair in range(d.NPAIR):
            gnext = gen_proj(pair + 1) if pair + 1 < d.NPAIR else None
            q_sb, k_sb = qk_tiles.pop(pair)
            # av[h01][qc]: [HD+1, QCW] accumulators (row HD = denominators)
            av = [[psAV.tile([HD + 1, d.QCW], F32, tag="av", name=f"av{h}{q}")
                   for q in range(d.QC)] for h in range(2)]
            p_tiles = {}  # (kt, h01) -> p tile [P, SQ]

            def emit_s(kt, h01):
                s01 = []
                for qc in range(d.QC):
                    s = psS.tile([P, d.QCW], F32, tag="ps", name=f"s{h01}{qc}")
                    nc.tensor.matmul(
                        s[:],
                        k_sb[h01 * HD:(h01 + 1) * HD, kt * P:(kt + 1) * P],
                        q_sb[h01 * HD:(h01 + 1) * HD, qc * d.QCW:(qc + 1) * d.QCW],
                        start=True, stop=True, tile_position=(h01 * HD, 0))
                    s01.append(s)
                p_sb = p_pool.tile([P, d.SQ], BF16, tag="p", name="p_sb")
                for qc in range(d.QC):
                    pv = p_sb[:, qc * d.QCW:(qc + 1) * d.QCW]
                    mv = mT[:, kt, qc * d.QCW:(qc + 1) * d.QCW]
                    if (kt + h01 + qc) % 2 == 0:
                        nc.scalar.activation(s01[qc][:], s01[qc][:], Exp)
                        nc.vector.tensor_tensor(pv, s01[qc][:], mv,
                                                mybir.AluOpType.mult)
                    else:
                        nc.scalar.activation(pv, s01[qc][:], Exp)
                        nc.gpsimd.tensor_tensor(pv, pv, mv,
                                                mybir.AluOpType.mult)
                p_tiles[(kt, h01)] = p_sb

            def emit_av(kt, h01):
                vh = pair * 2 + h01
                chunk, hh = divmod(vh, d.VH)
                vcol = chunk * d.VW + hh * (HD + 1)
                p_sb = p_tiles.pop((kt, h01))
                for qc in range(d.QC):
                    nc.tensor.matmul(
                        av[h01][qc][:], v_sb[:, kt, vcol:vcol + HD + 1],
                        p_sb[:, qc * d.QCW:(qc + 1) * d.QCW],
                        start=(kt == 0), stop=(kt == d.KT - 1))

            for kt in range(d.KT):
                emit_s(kt, 0)
                if kt >= LAG:
                    emit_av(kt - LAG, 0)
                emit_s(kt, 1)
                if kt >= LAG:
                    emit_av(kt - LAG, 1)
                if pend_norm is not None and kt in (1, 3, 5):
                    next(pend_norm, None)
                if gnext is not None and kt % 2 == 1:
                    next(gnext, None)
            for kt in range(d.KT - LAG, d.KT):
                emit_av(kt, 0)
                emit_av(kt, 1)
            if gnext is not None:
                for _ in gnext:
                    pass
            if pend_norm is not None:
                for _ in pend_norm:
                    pass
            pend_norm = norm_steps(pair, av)

        # last pair's normalization, then output projection
        for _ in pend_norm:
            pass

        # ---- output projection: yT = W_p^T ao^T + b_p, W_p stationary
        for ot in range(d.CT):
            o0 = ot * P
            psy = [psS.tile([P, d.QCW], F32, tag="ps", name=f"psy{qc}")
                   for qc in range(d.QC)]
            for ct in range(d.CT):
                for qc in range(d.QC):
                    nc.tensor.matmul(
                        psy[qc][:], wp_sb[:, ct, o0:o0 + P],
                        ao[:, ct, qc * d.QCW:(qc + 1) * d.QCW],
                        start=(ct == 0), stop=(ct == d.CT - 1))
            for qc in range(d.QC):
                y_sb = y_pool.tile([P, d.QCW], F32, tag="y")
                nc.vector.tensor_scalar_add(y_sb[:], psy[qc][:], bp_col[:, ot:ot + 1])
                nc.sync.dma_start(
                    out=yT_h[o0:o0 + P, qc * d.QCW:(qc + 1) * d.QCW], in_=y_sb[:])


def build_nc(d, rep=1):
    nc = bacc.Bacc(None)
    # ISA reports ~224KB/partition active SBUF but only ~208KB is usable on
    # this part; allocating above that wedges the core (observed on HW).
    nc.sbuf_top = min(nc.sbuf_top, 208 * 1024)
    xT_h = nc.dram_tensor("xT", [d.C, d.S], BF16, kind="ExternalInput")
    wqkT_h = nc.dram_tensor("wqkT", [d.C, 2 * d.C], BF16, kind="ExternalInput")
    wvT_h = nc.dram_tensor("wvT", [d.C, d.C], BF16, kind="ExternalInput")
    wpT_h = nc.dram_tensor("wpT", [d.C, d.C], BF16, kind="ExternalInput")
    bqk_h = nc.dram_tensor("bqk", [2 * d.C], F32, kind="ExternalInput")
    bv_h = nc.dram_tensor("bv", [d.C], F32, kind="ExternalInput")
    bp_h = nc.dram_tensor("bp", [d.C], F32, kind="ExternalInput")
    maskT_h = nc.dram_tensor("maskT", [d.S, d.SQ], BF16, kind="ExternalInput")
    yT_h = nc.dram_tensor("yT", [d.C, d.SQ], F32, kind="ExternalOutput")
    io = (xT_h, wqkT_h, wvT_h, wpT_h, bqk_h, bv_h, bp_h, maskT_h, yT_h)
    with tile.TileContext(nc) as tc:
        with ExitStack() as ctx:
            emit_body(ctx, tc, d, io, rep=rep)
    nc.compile()
    return nc


def to_bf16(a):
    import ml_dtypes
    return np.asarray(a, np.float32).astype(ml_dtypes.bfloat16)


def host_prep_core(d, x_b, mask_b, rq):
    """x_b [S, C] f32, mask_b [S(q?), S] bool (full batch mask), rq in {0,1}."""
    SQ = d.SQ
    perm = np.concatenate([np.arange(rq * SQ, (rq + 1) * SQ),
                           np.concatenate([np.arange(0, rq * SQ), np.arange((rq + 1) * SQ, d.S)])]).astype(np.int64)
    xT = to_bf16(np.ascontiguousarray(x_b.T[:, perm]))
    mq = ~mask_b[rq * SQ:(rq + 1) * SQ, :]          # keep-mask for our q rows
    maskT = to_bf16(np.ascontiguousarray(mq[:, perm].T))
    return xT, maskT


def host_prep_shared(d, w_qkv, b_qkv, w_proj, b_proj):
    C = d.C
    scale = np.float32(d.HD ** -0.5)
    wq = w_qkv[:C] * scale
    wk = w_qkv[C:2 * C]
    wqkT = to_bf16(np.ascontiguousarray(np.concatenate([wq, wk], axis=0).T, dtype=np.float32))
    wvT = to_bf16(np.ascontiguousarray(w_qkv[2 * C:].T, dtype=np.float32))
    wpT = to_bf16(np.ascontiguousarray(w_proj.T, dtype=np.float32))
    bqk = np.concatenate([b_qkv[:C] * scale, b_qkv[C:2 * C]]).astype(np.float32)
    bv = b_qkv[2 * C:].astype(np.float32)
    bp = b_proj.astype(np.float32)
    return wqkT, wvT, wpT, bqk, bv, bp


_NC_CACHE = {}


def kernel(x, w_qkv, b_qkv, w_proj, b_proj, attn_mask):
    from concourse.bass_utils import run_bass_kernel_spmd
    d = FULL
    B = x.shape[0]
    x = np.asarray(x, dtype=np.float32)
    attn_mask = np.asarray(attn_mask)
    wqkT, wvT, wpT, bqk, bv, bp = host_prep_shared(
        d, np.asarray(w_qkv, np.float32), np.asarray(b_qkv, np.float32),
        np.asarray(w_proj, np.float32), np.asarray(b_proj, np.float32))
    in_maps = []
    for c in range(8):
        b, rq = c // 2, c % 2
        xT, maskT = host_prep_core(d, x[b], np.asarray(attn_mask[b, 0], bool), rq)
        in_maps.append(dict(xT=xT, wqkT=wqkT, wvT=wvT, wpT=wpT, bqk=bqk, bv=bv,
                            bp=bp, maskT=maskT))
    if "nc" not in _NC_CACHE:
        _NC_CACHE["nc"] = build_nc(d, rep=1)
    nc = _NC_CACHE["nc"]
    res = run_bass_kernel_spmd(nc, in_maps, core_ids=list(range(8)))
    out = np.empty((B, d.S, d.C), np.float32)
    for c in range(8):
        b, rq = c // 2, c % 2
        out[b, rq * d.SQ:(rq + 1) * d.SQ] = np.ascontiguousarray(res.results[c]["yT"].T)
    return out

